# revision 7
# baseline (speedup 1.0000x reference)
"""Trainium2 Bass kernel for the Neural Tensor Layer problem.

Computes, for inputs
  state_a [B=64, S=64, H=512], state_b [B,S,H], M [F=32,H,H], V [2H,F=32],
  bias [B,S,F]:
    b_sum    = state_b.sum(axis=1)                       # [B, H]
    U        = einsum('fkh,bh->bfk', M, b_sum)           # [B, F, K]
    bilinear = einsum('bsk,bfk->bsf', state_a, U)
    v_term   = cat(state_a, state_b, -1) @ V             # [B, S, F]
    out      = sigmoid(tanh(bilinear + v_term + bias))

Sharding: 8 cores = 2 batch-groups (32 batches) x 4 feature-groups (8 feats).
Each core receives host-pretransposed shards and computes its [32, 64, 8]
slice of the output entirely on device.
"""

import sys

for _p in ("/opt/trn_rl_repo",):
    if _p not in sys.path:
        sys.path.insert(0, _p)

import numpy as np

import concourse.bass as bass
import concourse.mybir as mybir
from concourse.tile import TileContext
from concourse.bass_utils import run_bass_kernel_spmd
from concourse.masks import make_identity

B, S, H, F = 64, 64, 512, 32
N_CORES = 8
BG = 2          # batch groups
FG = 4          # feature groups
Bg = B // BG    # 32 batches per core
Fj = F // FG    # 8 features per core
KB = H // 128   # 4 contraction chunks of 128

F32 = mybir.dt.float32
F32R = mybir.dt.float32r


def _split_multi_waits(nc, max_waits=1):
    """The walrus pinned in this container encodes at most one sync-wait per
    instruction (CoreV3 setupSyncWait). Hoist extra waits onto same-engine
    NoOps placed immediately before the offending instruction."""
    for f in nc.m.functions:
        for bb in f.blocks:
            new_insts = []
            for inst in bb.instructions:
                si = inst.sync_info
                if si is not None and si.on_wait and len(si.on_wait) > max_waits:
                    waits = list(si.on_wait)
                    for w in waits[:-max_waits]:
                        nop = mybir.InstNoOp(
                            name=nc.get_next_instruction_name(),
                            text_hint="split_wait",
                            bass_nofuse=True,
                            engine=inst.engine,
                            sync_info=mybir.SyncInfo(on_wait=[w], on_update=[]),
                        )
                        nc.register_instruction(nop, overwrite=True)
                        new_insts.append(nop)
                    si.on_wait = waits[-max_waits:]
                new_insts.append(inst)
            bb.instructions = new_insts


def _build(use_f32r_stage2=False, use_f32r_v=False):
    nc = bass.Bass("TRN2", target_bir_lowering=False, debug=False,
                   num_devices=N_CORES)

    at_d = nc.declare_dram_parameter("at", [H, Bg * S], F32, isOutput=False)
    bt_d = nc.declare_dram_parameter("bt", [H, Bg * S], F32, isOutput=False)
    mt_d = nc.declare_dram_parameter("mt", [H, Fj * H], F32, isOutput=False)
    va_d = nc.declare_dram_parameter("va", [128, KB * Fj], F32, isOutput=False)
    vb_d = nc.declare_dram_parameter("vb", [128, KB * Fj], F32, isOutput=False)
    bias_d = nc.declare_dram_parameter("biasT", [Fj, Bg * S], F32, isOutput=False)
    out_d = nc.declare_dram_parameter("out", [Fj, Bg * S], F32, isOutput=True)

    def mmdt(ap, f32r):
        return ap.bitcast(F32R) if f32r else ap

    with TileContext(nc) as tc:
        with (
            tc.tile_pool(name="big", bufs=1) as big,        # resident inputs
            tc.tile_pool(name="small", bufs=1) as small,    # small resident
            tc.tile_pool(name="ufp", bufs=3) as ufp,        # U rows in sbuf
            tc.tile_pool(name="s2p", bufs=2, space="PSUM") as s2p,
            tc.tile_pool(name="trp", bufs=2, space="PSUM") as trp,
            tc.tile_pool(name="o3p", bufs=1, space="PSUM") as o3p,
            tc.tile_pool(name="outp", bufs=2) as outp,
        ):
            # ---- resident input tiles --------------------------------------
            bt_sb = []
            for i in range(KB):
                t = big.tile([128, Bg * S], F32, name=f"bt{i}", tag=f"bt{i}")
                nc.sync.dma_start(out=t[:], in_=bt_d[128 * i:128 * (i + 1), :])
                bt_sb.append(t)

            mt_sb = []
            for i in range(KB):
                t = big.tile([128, Fj * H], F32, name=f"mt{i}", tag=f"mt{i}")
                nc.sync.dma_start(out=t[:], in_=mt_d[128 * i:128 * (i + 1), :])
                mt_sb.append(t)

            at_sb = []
            for i in range(KB):
                t = big.tile([128, Bg * S], F32, name=f"at{i}", tag=f"at{i}")
                nc.sync.dma_start(out=t[:], in_=at_d[128 * i:128 * (i + 1), :])
                at_sb.append(t)

            va_sb = small.tile([128, KB * Fj], F32, name="va")
            nc.sync.dma_start(out=va_sb[:], in_=va_d[:])
            vb_sb = small.tile([128, KB * Fj], F32, name="vb")
            nc.sync.dma_start(out=vb_sb[:], in_=vb_d[:])
            bias_sb = small.tile([Fj, Bg * S], F32, name="bias")
            nc.sync.dma_start(out=bias_sb[:], in_=bias_d[:])

            ident = small.tile([Bg, Bg], F32, name="ident")
            make_identity(nc, ident[:])

            # ---- stage 1: b_sumT[h, b] = sum_s bt[h, b, s] ------------------
            bsumT = []
            for i in range(KB):
                t = small.tile([128, Bg], F32, name=f"bsum{i}", tag=f"bsum{i}")
                nc.vector.reduce_sum(
                    t[:], bt_sb[i].rearrange("p (b s) -> p b s", s=S),
                    axis=mybir.AxisListType.X)
                bsumT.append(t)

            # ---- stage 3 v-part: O_t[f, (b s)] += V.T @ states --------------
            # (independent of stages 2/2.5; only needs bt/at + V slices)
            o_ps = []
            for t_i in range(KB):  # 4 groups of 8 batches
                o = o3p.tile([Fj, 8 * S], F32, name=f"o{t_i}", tag=f"o{t_i}")
                cols = slice(t_i * 8 * S, (t_i + 1) * 8 * S)
                # va is folded into uall during stage 2.5, so the v-term here
                # only needs the state_b half of cat(a, b) @ V.
                for kb in range(KB):
                    nc.tensor.matmul(
                        o[:],
                        mmdt(vb_sb[:, kb * Fj:(kb + 1) * Fj], use_f32r_v),
                        mmdt(bt_sb[kb][:, cols], use_f32r_v),
                        start=(kb == 0), stop=False)
                o_ps.append(o)

            # ---- stage 2: U[b, f, k] accumulated over h chunks -------------
            # P_q [32 b, 512 k] for each local feature q; then transpose to
            # k-major and fold in va: Uall_kb[k, b*Fj + f] = U + va[k, f].
            uall = []
            for kb in range(KB):
                t = small.tile([128, Bg * Fj], F32, name=f"uall{kb}",
                               tag=f"uall{kb}")
                uall.append(t)

            for q in range(Fj):
                p_q = s2p.tile([Bg, H], F32, name=f"pq{q}", tag="pq")
                for i in range(KB):
                    nc.tensor.matmul(
                        p_q[:],
                        mmdt(bsumT[i][:], use_f32r_stage2),
                        mmdt(mt_sb[i][:, q * H:(q + 1) * H], use_f32r_stage2),
                        start=(i == 0), stop=(i == KB - 1))
                # PSUM -> SBUF (transpose input must be SBUF)
                u_q = ufp.tile([Bg, H], F32, name="u_q", tag="u_q")
                nc.vector.tensor_copy(u_q[:], p_q[:])
                for kb in range(KB):
                    t_ps = trp.tile([128, Bg], F32, name="t_ps", tag="t_ps")
                    nc.tensor.transpose(
                        t_ps[:], u_q[:, kb * 128:(kb + 1) * 128], ident[:])
                    dst = uall[kb].rearrange("p (b f) -> p b f", f=Fj)[:, :, q]
                    nc.vector.tensor_scalar_add(
                        dst, t_ps[:], va_sb[:, kb * Fj + q:kb * Fj + q + 1])

            # ---- stage 3 bilinear: O_t[f, b_local*S + s] += U'_b.T @ a_b ----
            for t_i in range(KB):
                o = o_ps[t_i]
                for b_local in range(8):
                    b = t_i * 8 + b_local
                    for kb in range(KB):
                        nc.tensor.matmul(
                            o[:, b_local * S:(b_local + 1) * S],
                            uall[kb][:, b * Fj:(b + 1) * Fj],
                            at_sb[kb][:, b * S:(b + 1) * S],
                            start=False,
                            stop=(kb == KB - 1))

            # ---- epilogue: sigmoid(tanh(O + bias)) -> DRAM ------------------
            out_sb = outp.tile([Fj, Bg * S], F32, name="out_sb", tag="out_sb",
                               bufs=1)
            for t_i in range(KB):
                cols = slice(t_i * 8 * S, (t_i + 1) * 8 * S)
                tmp = outp.tile([Fj, 8 * S], F32, name="tmp", tag="tmp")
                nc.vector.tensor_tensor(
                    out=tmp[:], in0=o_ps[t_i][:], in1=bias_sb[:, cols],
                    op=mybir.AluOpType.add)
                nc.scalar.activation(tmp[:], tmp[:],
                                     mybir.ActivationFunctionType.Tanh)
                nc.scalar.activation(out_sb[:, cols], tmp[:],
                                     mybir.ActivationFunctionType.Sigmoid)
            nc.sync.dma_start(out=out_d[:], in_=out_sb[:])

    _split_multi_waits(nc)
    return nc


_NC_CACHE = {}


def _get_nc(key=("f32", "f32")):
    if key not in _NC_CACHE:
        _NC_CACHE[key] = _build(
            use_f32r_stage2=(key[0] == "f32r"), use_f32r_v=(key[1] == "f32r"))
    return _NC_CACHE[key]


def kernel(state_a, state_b, M, V, bias):
    state_a = np.asarray(state_a, dtype=np.float32)
    state_b = np.asarray(state_b, dtype=np.float32)
    M = np.asarray(M, dtype=np.float32)
    V = np.asarray(V, dtype=np.float32)
    bias = np.asarray(bias, dtype=np.float32)

    nc = _get_nc()

    in_maps = []
    for c in range(N_CORES):
        g, j = divmod(c, FG)
        bs = slice(g * Bg, (g + 1) * Bg)
        fs = slice(j * Fj, (j + 1) * Fj)
        at = np.ascontiguousarray(
            state_a[bs].transpose(2, 0, 1).reshape(H, Bg * S))
        bt = np.ascontiguousarray(
            state_b[bs].transpose(2, 0, 1).reshape(H, Bg * S))
        mt = np.ascontiguousarray(
            M[fs].transpose(2, 0, 1).reshape(H, Fj * H))
        va = np.ascontiguousarray(
            V[:H, fs].reshape(KB, 128, Fj).transpose(1, 0, 2).reshape(128, KB * Fj))
        vb = np.ascontiguousarray(
            V[H:, fs].reshape(KB, 128, Fj).transpose(1, 0, 2).reshape(128, KB * Fj))
        biasT = np.ascontiguousarray(
            bias[bs, :, fs].transpose(2, 0, 1).reshape(Fj, Bg * S))
        in_maps.append(
            {"at": at, "bt": bt, "mt": mt, "va": va, "vb": vb, "biasT": biasT})

    res = run_bass_kernel_spmd(nc, in_maps, list(range(N_CORES)))

    out = np.empty((B, S, F), dtype=np.float32)
    for c in range(N_CORES):
        g, j = divmod(c, FG)
        o = res.results[c]["out"].reshape(Fj, Bg, S).transpose(1, 2, 0)
        out[g * Bg:(g + 1) * Bg, :, j * Fj:(j + 1) * Fj] = o
    return out
